# revision 33
# baseline (speedup 1.0000x reference)
"""Batched rule-expert FFN (MoE routing) on 8 Trainium2 NeuronCores.

Strategy (expert/slot parallel with host-side dispatch):
  - Sort tokens by rule id on the host; each rule's tokens form one "slot"
    (rules with more than `Cr` tokens get several slots, zero-hit rules get
    an empty slot so the device schedule stays fully static).
  - Slots are dealt contiguously to the 8 cores (128 slots/core for 1024
    rules).  For each core the host gathers that core's slot weights into
    block-major, DMA-contiguous device layouts, plus an x^T buffer whose
    column block k*Cr:(k+1)*Cr holds the (transposed, zero-padded) tokens
    of slot k.
  - The device kernel is a static loop over slot-blocks of B rules:
      psum_h(c)  = bias1(c) x selector  (one K=B seeding matmul per chunk)
                 + per-rule W1(c)^T X^T  matmuls        [128, B*Cr]
      H(c)       = gelu(psum_h(c))      (one big ACT op per chunk)
      psum_o     = bias2 x selector + per-rule W2^T H   [128, B*Cr]
      out        = copy(psum_o) -> DMA
  - Host scatters Out^T columns back to token order.

Traffic per core ~= w1+w2 tables for its 128 slots (16 MiB bf16 / 32 MiB
f32) + ~2 MiB x/out, which is the memory roofline: nearly every rule is
hit, so the whole table must be read exactly once.
"""

import numpy as np

import concourse.bacc as bacc
import concourse.mybir as mybir
from concourse.tile import TileContext
from concourse.bass_utils import run_bass_kernel_spmd

N_CORES = 8
D = 128   # d_model
E = 256   # expert dim
EC = E // 128   # e-chunks of 128 partitions
NB = 3          # bias chunks per rule (b1c0, b1c1, b2)

MM_DT = "float32"  # matmul-operand dtype: "float32" or "bfloat16"


def _build_nc(K_c: int, Cr: int, B: int, mm_dt: str = "float32"):
    """Bass program for one core: K_c slots of capacity Cr, B slots/block."""
    f32 = mybir.dt.float32
    nc = bacc.Bacc("TRN2", target_bir_lowering=False)

    mdt = {"bfloat16": mybir.dt.bfloat16,
           "float16": mybir.dt.float16}.get(mm_dt, f32)

    nblk = K_c // B
    W = B * Cr  # columns per block
    gelu = mybir.ActivationFunctionType.Gelu
    import os as _os
    DW = int(_os.environ.get("DW_BLKS", "2"))
    if nblk % DW:
        DW = 1

    xT = nc.declare_dram_parameter("xT", [nblk, D, W], mdt, isOutput=False)
    w1 = nc.declare_dram_parameter(
        "w1", [nblk // DW, D, DW * B * E], mdt, isOutput=False)
    w2 = nc.declare_dram_parameter(
        "w2", [nblk // DW, 128, DW * B * EC * D], mdt, isOutput=False)
    bb = nc.declare_dram_parameter(
        "bias", [nblk, B, NB * 128], mdt, isOutput=False)
    seld = nc.declare_dram_parameter("sel", [B, W], mdt, isOutput=False)
    outT = nc.declare_dram_parameter("outT", [nblk, D, W], f32, isOutput=True)

    with TileContext(nc) as tc:
        with (
            tc.tile_pool(name="wpool", bufs=3) as wpool,
            tc.tile_pool(name="xpool", bufs=3) as xpool,
            tc.tile_pool(name="bpool", bufs=3) as bpool,
            tc.tile_pool(name="hpool", bufs=3) as hpool,
            tc.tile_pool(name="opool", bufs=3) as opool,
            tc.tile_pool(name="spool", bufs=1) as spool,
            tc.tile_pool(name="ppool", bufs=2, space="PSUM") as ppool,
        ):
            # block-identity selector: sel[b, b*Cr:(b+1)*Cr] = 1, else 0
            sel = spool.tile([B, W], mdt)
            nc.sync.dma_start(out=sel, in_=seld[:])

            w1w = w2w = None
            for j in range(nblk):
                if j % DW == 0:
                    w1w = wpool.tile([128, DW * B * E], mdt, tag="w1t")
                    nc.sync.dma_start(out=w1w, in_=w1[j // DW])
                    w2w = wpool.tile([128, DW * B * E], mdt, tag="w2t")
                    nc.scalar.dma_start(out=w2w, in_=w2[j // DW])
                hh = (j % DW) * B * E
                w1t = w1w[:, hh:hh + B * E]
                w2t = w2w[:, hh:hh + B * E]
                bt = bpool.tile([B, NB * 128], mdt, tag="bt")
                nc.gpsimd.dma_start(out=bt, in_=bb[j])
                xt = xpool.tile([128, W], mdt, tag="xt")
                nc.sync.dma_start(out=xt, in_=xT[j])

                ph0 = ppool.tile([128, W], f32, tag="ph0")
                ph1 = ppool.tile([128, W], f32, tag="ph1")
                po = ppool.tile([128, W], f32, tag="po", bufs=4)
                h0 = hpool.tile([128, W], mdt, tag="h0")
                h1 = hpool.tile([128, W], mdt, tag="h1")
                osb = opool.tile([128, W], f32, tag="osb")

                # ---- layer 1: H^T[e, tok] = gelu(W1^T X^T + b1) ---------
                nc.tensor.matmul(ph0, lhsT=bt[:, 0:128], rhs=sel,
                                 start=True, stop=False,
                                 skip_group_check=True)
                nc.tensor.matmul(ph1, lhsT=bt[:, 128:256], rhs=sel,
                                 start=True, stop=False,
                                 skip_group_check=True)
                for b in range(B):
                    cs = slice(b * Cr, (b + 1) * Cr)
                    nc.tensor.matmul(
                        ph0[:, cs], lhsT=w1t[:, b * E:b * E + 128],
                        rhs=xt[:, cs], start=False, stop=True,
                        skip_group_check=True)
                    nc.tensor.matmul(
                        ph1[:, cs], lhsT=w1t[:, b * E + 128:b * E + 256],
                        rhs=xt[:, cs], start=False, stop=True,
                        skip_group_check=True)
                nc.scalar.activation(h0, ph0, gelu)
                nc.scalar.activation(h1, ph1, gelu)

                # ---- layer 2: Out^T[d, tok] = W2^T H^T + b2 -------------
                nc.tensor.matmul(po, lhsT=bt[:, 256:384], rhs=sel,
                                 start=True, stop=False,
                                 skip_group_check=True)
                for b in range(B):
                    cs = slice(b * Cr, (b + 1) * Cr)
                    nc.tensor.matmul(
                        po[:, cs], lhsT=w2t[:, b * E:b * E + 128],
                        rhs=h0[:, cs], start=False, stop=False,
                        skip_group_check=True)
                    nc.tensor.matmul(
                        po[:, cs], lhsT=w2t[:, b * E + 128:b * E + 256],
                        rhs=h1[:, cs], start=False, stop=True,
                        skip_group_check=True)

                nc.vector.tensor_copy(osb, po)
                nc.sync.dma_start(out=outT[j], in_=osb)

    nc.compile()
    return nc


def _plan(rules: np.ndarray, R: int):
    """Sort tokens by rule, build fixed-capacity slots, deal to cores."""
    order = np.argsort(rules, kind="stable")
    counts = np.bincount(rules, minlength=R)
    starts = np.concatenate([[0], np.cumsum(counts)])

    Cr = int(max(8, counts.max()))
    Cr = (Cr + 3) // 4 * 4
    Cr = min(Cr, 512)
    import os
    pref = os.environ.get("PLAN_B")
    prefs = (int(pref),) if pref else (16, 8, 4, 2, 1)
    B = 1
    for Bc in prefs:
        if Bc * Cr <= 512:
            B = Bc
            break

    slots = []  # (sorted_start, length)
    for r in range(R):
        c = int(counts[r])
        s = int(starts[r])
        if c == 0:
            slots.append((s, 0))
        else:
            off = 0
            while off < c:
                ln = min(Cr, c - off)
                slots.append((s + off, ln))
                off += ln
    # rule id per slot for the weight gather
    slot_rules = []
    for r in range(R):
        c = int(counts[r])
        n = max(1, -(-c // Cr))
        slot_rules.extend([r] * n)

    S = len(slots)
    K_c = -(-S // (N_CORES * B)) * B  # slots per core, multiple of B
    total = K_c * N_CORES
    slots += [(0, 0)] * (total - S)
    slot_rules += [0] * (total - S)
    return order, np.array(slot_rules), slots, K_c, Cr, B


def _prepare(x, rules, w1, b1, w2, b2, mm_dt=MM_DT):
    x = np.ascontiguousarray(np.asarray(x), dtype=np.float32)
    rules = np.asarray(rules).astype(np.int64)
    w1 = np.ascontiguousarray(np.asarray(w1), dtype=np.float32)
    b1 = np.ascontiguousarray(np.asarray(b1), dtype=np.float32)
    w2 = np.ascontiguousarray(np.asarray(w2), dtype=np.float32)
    b2 = np.ascontiguousarray(np.asarray(b2), dtype=np.float32)

    R = w1.shape[0]
    order, slot_rules, slots, K_c, Cr, B = _plan(rules, R)
    nblk = K_c // B
    W = B * Cr

    if mm_dt == "bfloat16":
        import ml_dtypes
        mnp = ml_dtypes.bfloat16
    elif mm_dt == "float16":
        mnp = np.float16
    else:
        mnp = np.float32

    bcat = np.concatenate([b1, b2], axis=1)  # [R, E+D] = [R, NB*128]
    import os as _os
    DW = int(_os.environ.get("DW_BLKS", "2"))
    if nblk % DW:
        DW = 1

    def _regroup(a):
        # [nblk, P, C] -> [nblk//DW, P, DW*C] keeping per-partition rows
        # of DW consecutive blocks contiguous in DRAM
        n, P, C = a.shape
        return np.ascontiguousarray(
            a.reshape(n // DW, DW, P, C).swapaxes(1, 2)
            .reshape(n // DW, P, DW * C))

    in_maps = []
    for c in range(N_CORES):
        sl = slice(c * K_c, (c + 1) * K_c)
        sr = slot_rules[sl]
        xT = np.zeros((D, K_c * Cr), dtype=mnp)
        for k, (s, ln) in enumerate(slots[sl.start:sl.stop]):
            if ln:
                xT[:, k * Cr:k * Cr + ln] = x[order[s:s + ln]].T.astype(mnp)
        w1g = w1[sr].astype(mnp)  # [K_c, D, E]
        w2g = w2[sr].astype(mnp)  # [K_c, E, D]
        bg = bcat[sr].astype(mnp)  # [K_c, NB*128]
        selm = np.zeros((B, W), dtype=mnp)
        for b in range(B):
            selm[b, b * Cr:(b + 1) * Cr] = 1
        in_maps.append({
            "sel": selm,
            "xT": np.ascontiguousarray(
                xT.reshape(D, nblk, W).transpose(1, 0, 2)),
            "w1": _regroup(
                w1g.reshape(nblk, B, D, E).transpose(0, 2, 1, 3)
                .reshape(nblk, D, B * E)),
            "w2": _regroup(
                w2g.reshape(nblk, B, EC, 128, D).transpose(0, 3, 1, 2, 4)
                .reshape(nblk, 128, B * EC * D)),
            "bias": np.ascontiguousarray(bg.reshape(nblk, B, NB * 128)),
        })
    return in_maps, order, slots, K_c, Cr, B


def _unpack(res, order, slots, K_c, Cr, N, B):
    out = np.empty((N, D), dtype=np.float32)
    W = B * Cr
    for c in range(N_CORES):
        outT = res.results[c]["outT"]  # [nblk, D, W]
        o2 = outT.transpose(1, 0, 2).reshape(D, K_c * Cr)
        for k, (s, ln) in enumerate(slots[c * K_c:(c + 1) * K_c]):
            if ln:
                out[order[s:s + ln]] = o2[:, k * Cr:k * Cr + ln].T
    return out


def kernel(x, rules, w1, b1, w2, b2):
    N = np.asarray(x).shape[0]
    in_maps, order, slots, K_c, Cr, B = _prepare(
        x, rules, w1, b1, w2, b2, mm_dt=MM_DT)
    nc = _build_nc(K_c, Cr, B, mm_dt=MM_DT)
    res = run_bass_kernel_spmd(nc, in_maps, list(range(N_CORES)))
    return _unpack(res, order, slots, K_c, Cr, N, B)


# revision 34
# speedup vs baseline: 1.2840x; 1.2840x over previous
"""Batched rule-expert FFN (MoE routing) on 8 Trainium2 NeuronCores.

Strategy (expert/slot parallel with host-side dispatch):
  - Sort tokens by rule id on the host; each rule's tokens form one "slot"
    (rules with more than `Cr` tokens get several slots, zero-hit rules get
    an empty slot so the device schedule stays fully static).
  - Slots are dealt contiguously to the 8 cores (128 slots/core for 1024
    rules).  For each core the host gathers that core's slot weights into
    block-major, DMA-contiguous device layouts, plus an x^T buffer whose
    column block k*Cr:(k+1)*Cr holds the (transposed, zero-padded) tokens
    of slot k.
  - The device kernel is a static loop over slot-blocks of B rules:
      psum_h(c)  = bias1(c) x selector  (one K=B seeding matmul per chunk)
                 + per-rule W1(c)^T X^T  matmuls        [128, B*Cr]
      H(c)       = gelu(psum_h(c))      (one big ACT op per chunk)
      psum_o     = bias2 x selector + per-rule W2^T H   [128, B*Cr]
      out        = copy(psum_o) -> DMA
  - Host scatters Out^T columns back to token order.

Traffic per core ~= w1+w2 tables for its 128 slots (16 MiB bf16 / 32 MiB
f32) + ~2 MiB x/out, which is the memory roofline: nearly every rule is
hit, so the whole table must be read exactly once.
"""

import numpy as np

import concourse.bacc as bacc
import concourse.mybir as mybir
from concourse.tile import TileContext
from concourse.bass_utils import run_bass_kernel_spmd

N_CORES = 8
D = 128   # d_model
E = 256   # expert dim
EC = E // 128   # e-chunks of 128 partitions
NB = 3          # bias chunks per rule (b1c0, b1c1, b2)

MM_DT = "float32"  # matmul-operand dtype: "float32" or "bfloat16"


def _build_nc(K_c: int, Cr: int, B: int, mm_dt: str = "float32"):
    """Bass program for one core: K_c slots of capacity Cr, B slots/block."""
    f32 = mybir.dt.float32
    nc = bacc.Bacc("TRN2", target_bir_lowering=False)

    mdt = {"bfloat16": mybir.dt.bfloat16,
           "float16": mybir.dt.float16}.get(mm_dt, f32)

    nblk = K_c // B
    W = B * Cr  # columns per block
    gelu = mybir.ActivationFunctionType.Gelu
    import os as _os
    DW = int(_os.environ.get("DW_BLKS", "2"))
    if nblk % DW:
        DW = 1

    xT = nc.declare_dram_parameter("xT", [nblk, D, W], mdt, isOutput=False)
    w1 = nc.declare_dram_parameter(
        "w1", [nblk // DW, D, DW * B * E], mdt, isOutput=False)
    w2 = nc.declare_dram_parameter(
        "w2", [nblk // DW, 128, DW * B * EC * D], mdt, isOutput=False)
    bb = nc.declare_dram_parameter(
        "bias", [nblk, B, NB * 128], mdt, isOutput=False)
    seld = nc.declare_dram_parameter("sel", [B, W], mdt, isOutput=False)
    odt = mdt if mm_dt == "float16" else f32
    outT = nc.declare_dram_parameter("outT", [nblk, D, W], odt, isOutput=True)

    with TileContext(nc) as tc:
        with (
            tc.tile_pool(name="wpool", bufs=3) as wpool,
            tc.tile_pool(name="xpool", bufs=3) as xpool,
            tc.tile_pool(name="bpool", bufs=3) as bpool,
            tc.tile_pool(name="hpool", bufs=3) as hpool,
            tc.tile_pool(name="opool", bufs=3) as opool,
            tc.tile_pool(name="spool", bufs=1) as spool,
            tc.tile_pool(name="ppool", bufs=2, space="PSUM") as ppool,
        ):
            # block-identity selector: sel[b, b*Cr:(b+1)*Cr] = 1, else 0
            sel = spool.tile([B, W], mdt)
            nc.sync.dma_start(out=sel, in_=seld[:])

            w1w = w2w = None
            for j in range(nblk):
                if j % DW == 0:
                    w1w = wpool.tile([128, DW * B * E], mdt, tag="w1t")
                    nc.sync.dma_start(out=w1w, in_=w1[j // DW])
                    w2w = wpool.tile([128, DW * B * E], mdt, tag="w2t")
                    nc.scalar.dma_start(out=w2w, in_=w2[j // DW])
                hh = (j % DW) * B * E
                w1t = w1w[:, hh:hh + B * E]
                w2t = w2w[:, hh:hh + B * E]
                bt = bpool.tile([B, NB * 128], mdt, tag="bt")
                nc.gpsimd.dma_start(out=bt, in_=bb[j])
                xt = xpool.tile([128, W], mdt, tag="xt")
                nc.sync.dma_start(out=xt, in_=xT[j])

                ph0 = ppool.tile([128, W], f32, tag="ph0")
                ph1 = ppool.tile([128, W], f32, tag="ph1")
                po = ppool.tile([128, W], f32, tag="po", bufs=4)
                h0 = hpool.tile([128, W], mdt, tag="h0")
                h1 = hpool.tile([128, W], mdt, tag="h1")
                osb = opool.tile([128, W], odt, tag="osb")

                # ---- layer 1: H^T[e, tok] = gelu(W1^T X^T + b1) ---------
                nc.tensor.matmul(ph0, lhsT=bt[:, 0:128], rhs=sel,
                                 start=True, stop=False,
                                 skip_group_check=True)
                nc.tensor.matmul(ph1, lhsT=bt[:, 128:256], rhs=sel,
                                 start=True, stop=False,
                                 skip_group_check=True)
                for b in range(B):
                    cs = slice(b * Cr, (b + 1) * Cr)
                    nc.tensor.matmul(
                        ph0[:, cs], lhsT=w1t[:, b * E:b * E + 128],
                        rhs=xt[:, cs], start=False, stop=True,
                        skip_group_check=True)
                    nc.tensor.matmul(
                        ph1[:, cs], lhsT=w1t[:, b * E + 128:b * E + 256],
                        rhs=xt[:, cs], start=False, stop=True,
                        skip_group_check=True)
                nc.scalar.activation(h0, ph0, gelu)
                nc.scalar.activation(h1, ph1, gelu)

                # ---- layer 2: Out^T[d, tok] = W2^T H^T + b2 -------------
                nc.tensor.matmul(po, lhsT=bt[:, 256:384], rhs=sel,
                                 start=True, stop=False,
                                 skip_group_check=True)
                for b in range(B):
                    cs = slice(b * Cr, (b + 1) * Cr)
                    nc.tensor.matmul(
                        po[:, cs], lhsT=w2t[:, b * E:b * E + 128],
                        rhs=h0[:, cs], start=False, stop=False,
                        skip_group_check=True)
                    nc.tensor.matmul(
                        po[:, cs], lhsT=w2t[:, b * E + 128:b * E + 256],
                        rhs=h1[:, cs], start=False, stop=True,
                        skip_group_check=True)

                nc.vector.tensor_copy(osb, po)
                nc.sync.dma_start(out=outT[j], in_=osb)

    nc.compile()
    return nc


def _plan(rules: np.ndarray, R: int):
    """Sort tokens by rule, build fixed-capacity slots, deal to cores."""
    order = np.argsort(rules, kind="stable")
    counts = np.bincount(rules, minlength=R)
    starts = np.concatenate([[0], np.cumsum(counts)])

    Cr = int(max(8, counts.max()))
    Cr = (Cr + 3) // 4 * 4
    Cr = min(Cr, 512)
    import os
    pref = os.environ.get("PLAN_B")
    prefs = (int(pref),) if pref else (16, 8, 4, 2, 1)
    B = 1
    for Bc in prefs:
        if Bc * Cr <= 512:
            B = Bc
            break

    slots = []  # (sorted_start, length)
    for r in range(R):
        c = int(counts[r])
        s = int(starts[r])
        if c == 0:
            slots.append((s, 0))
        else:
            off = 0
            while off < c:
                ln = min(Cr, c - off)
                slots.append((s + off, ln))
                off += ln
    # rule id per slot for the weight gather
    slot_rules = []
    for r in range(R):
        c = int(counts[r])
        n = max(1, -(-c // Cr))
        slot_rules.extend([r] * n)

    S = len(slots)
    K_c = -(-S // (N_CORES * B)) * B  # slots per core, multiple of B
    total = K_c * N_CORES
    slots += [(0, 0)] * (total - S)
    slot_rules += [0] * (total - S)
    return order, np.array(slot_rules), slots, K_c, Cr, B


def _prepare(x, rules, w1, b1, w2, b2, mm_dt=MM_DT):
    x = np.ascontiguousarray(np.asarray(x), dtype=np.float32)
    rules = np.asarray(rules).astype(np.int64)
    w1 = np.ascontiguousarray(np.asarray(w1), dtype=np.float32)
    b1 = np.ascontiguousarray(np.asarray(b1), dtype=np.float32)
    w2 = np.ascontiguousarray(np.asarray(w2), dtype=np.float32)
    b2 = np.ascontiguousarray(np.asarray(b2), dtype=np.float32)

    R = w1.shape[0]
    order, slot_rules, slots, K_c, Cr, B = _plan(rules, R)
    nblk = K_c // B
    W = B * Cr

    if mm_dt == "bfloat16":
        import ml_dtypes
        mnp = ml_dtypes.bfloat16
    elif mm_dt == "float16":
        mnp = np.float16
    else:
        mnp = np.float32

    bcat = np.concatenate([b1, b2], axis=1)  # [R, E+D] = [R, NB*128]
    import os as _os
    DW = int(_os.environ.get("DW_BLKS", "2"))
    if nblk % DW:
        DW = 1

    def _regroup(a):
        # [nblk, P, C] -> [nblk//DW, P, DW*C] keeping per-partition rows
        # of DW consecutive blocks contiguous in DRAM
        n, P, C = a.shape
        return np.ascontiguousarray(
            a.reshape(n // DW, DW, P, C).swapaxes(1, 2)
            .reshape(n // DW, P, DW * C))

    in_maps = []
    for c in range(N_CORES):
        sl = slice(c * K_c, (c + 1) * K_c)
        sr = slot_rules[sl]
        xT = np.zeros((D, K_c * Cr), dtype=mnp)
        for k, (s, ln) in enumerate(slots[sl.start:sl.stop]):
            if ln:
                xT[:, k * Cr:k * Cr + ln] = x[order[s:s + ln]].T.astype(mnp)
        w1g = w1[sr].astype(mnp)  # [K_c, D, E]
        w2g = w2[sr].astype(mnp)  # [K_c, E, D]
        bg = bcat[sr].astype(mnp)  # [K_c, NB*128]
        selm = np.zeros((B, W), dtype=mnp)
        for b in range(B):
            selm[b, b * Cr:(b + 1) * Cr] = 1
        in_maps.append({
            "sel": selm,
            "xT": np.ascontiguousarray(
                xT.reshape(D, nblk, W).transpose(1, 0, 2)),
            "w1": _regroup(
                w1g.reshape(nblk, B, D, E).transpose(0, 2, 1, 3)
                .reshape(nblk, D, B * E)),
            "w2": _regroup(
                w2g.reshape(nblk, B, EC, 128, D).transpose(0, 3, 1, 2, 4)
                .reshape(nblk, 128, B * EC * D)),
            "bias": np.ascontiguousarray(bg.reshape(nblk, B, NB * 128)),
        })
    return in_maps, order, slots, K_c, Cr, B


def _unpack(res, order, slots, K_c, Cr, N, B):
    out = np.empty((N, D), dtype=np.float32)
    W = B * Cr
    for c in range(N_CORES):
        outT = res.results[c]["outT"]  # [nblk, D, W]
        o2 = outT.transpose(1, 0, 2).reshape(D, K_c * Cr)
        for k, (s, ln) in enumerate(slots[c * K_c:(c + 1) * K_c]):
            if ln:
                out[order[s:s + ln]] = o2[:, k * Cr:k * Cr + ln].T
    return out


def kernel(x, rules, w1, b1, w2, b2):
    N = np.asarray(x).shape[0]
    in_maps, order, slots, K_c, Cr, B = _prepare(
        x, rules, w1, b1, w2, b2, mm_dt=MM_DT)
    nc = _build_nc(K_c, Cr, B, mm_dt=MM_DT)
    res = run_bass_kernel_spmd(nc, in_maps, list(range(N_CORES)))
    return _unpack(res, order, slots, K_c, Cr, N, B)
